# revision 3
# baseline (speedup 1.0000x reference)
"""Trainium2 Bass kernel for nn_Affine_Linear_22067541967103.

Math (per point p = (b, n, d), vectors in R^3):
    a1 = J[p,:,0], a2 = J[p,:,1], x = X[p,:]
    The Gram-Schmidt frame (b1,b2,b3) is orthonormal, so the reference
    reduces to (b3 = normalize(a1 x a2)):
        c_term = b3 (b3 . x)
        b_term = b3 x x
        a_term = x - c_term
    Y[b,n] = A @ X + (C-A) @ c_term + Bm @ b_term     (contraction over d)

Device computation per point:
    c  = a1 x a2            (fp32, cancellation-safe)
    s  = |c|^2, q = 1/s, r = sqrt(q)
    cr = c * r  (= b3), H[j,k] = cr_j * x_k, t = sum_k H[k,k] (= b3 . x)
    ct = cr * t
    b_term_i = H[a,b] - H[b,a] is folded into the matmul via +Bm / -Bm.

Layout: core c handles batch b=c. Partition p = h*64 + d (h = n-half),
free axis j: n = h*4096 + j. All per-component planes are [128, 4096].
"""

import numpy as np

B, N, D, F = 8, 8192, 64, 64
NCORES = 8
NHALF = N // 2           # 4096 free columns per core
T = 1024                 # chunk columns
NCH = NHALF // T
MM_FREE = 512            # PSUM bank free-dim limit (fp32)

# J plane order in DRAM: [a1_1, a1_2, a1_0, a2_2, a2_0, a2_1]
# so that pa_k = a1_{k+1} * a2_{k+2} is one fused [128,3,T] multiply.
_J_PLANES = [(1, 0), (2, 0), (0, 0), (2, 1), (0, 1), (1, 1)]  # (k, c)

_CACHE: dict = {}


def _build_nc():
    import concourse.bacc as bacc
    import concourse.bass as bass
    import concourse.tile as tile
    import concourse.mybir as mybir
    from contextlib import ExitStack

    dt = mybir.dt
    nc = bacc.Bacc("TRN2", target_bir_lowering=False, debug=False, num_devices=NCORES)

    jt = nc.dram_tensor("jt", [6, 128, NHALF], dt.float32, kind="ExternalInput")
    xt = nc.dram_tensor("xt", [3, 128, NHALF], dt.float16, kind="ExternalInput")
    wt = nc.dram_tensor("wt", [4, 128, 128], dt.float16, kind="ExternalInput")
    yt = nc.dram_tensor("yt", [3, 128, NHALF], dt.float16, kind="ExternalOutput")

    AF = mybir.ActivationFunctionType
    # (a, b) pairs per output component i: bt_i = H[a][b] - H[b][a]
    AB = [(1, 2), (2, 0), (0, 1)]
    # pb_k = a1_{k+2} * a2_{k+1}: slot pairs in jin (see _J_PLANES):
    # a1_{k+2} -> slots (1, 2, 0); a2_{k+1} -> slots (5, 3, 4)
    PB_SLOTS = [(1, 5), (2, 3), (0, 4)]

    with tile.TileContext(nc) as tc, ExitStack() as ctx:
        pool = ctx.enter_context(tc.tile_pool(name="main", bufs=1))
        psum = ctx.enter_context(tc.tile_pool(name="psum", bufs=1, space="PSUM"))

        wtile = pool.tile([128, 4, 128], dt.float16, tag="wt", bufs=1)
        nc.sync.dma_start(wtile[:], wt.ap().rearrange("w p m -> p w m"))

        for ch in range(NCH):
            j0 = ch * T
            cols = slice(j0, j0 + T)

            jin = pool.tile([128, 6, T], dt.float32, tag="jin", bufs=2, name=f"jin{ch}")
            nc.sync.dma_start(jin[:], jt.ap()[:, :, cols].rearrange("m p t -> p m t"))
            xin = pool.tile([128, 3, T], dt.float16, tag="xin", bufs=2, name=f"xin{ch}")
            nc.sync.dma_start(xin[:], xt.ap()[:, :, cols].rearrange("m p t -> p m t"))

            # cross products (POOL), fp32 for cancellation safety
            pa = pool.tile([128, 3, T], dt.float32, tag="pa", bufs=1, name=f"pa{ch}")
            nc.gpsimd.tensor_mul(pa[:], jin[:, 0:3, :], jin[:, 3:6, :])
            pb = pool.tile([128, 3, T], dt.float32, tag="pb", bufs=1, name=f"pb{ch}")
            for k, (s1, s2) in enumerate(PB_SLOTS):
                nc.gpsimd.tensor_mul(pb[:, k, :], jin[:, s1, :], jin[:, s2, :])

            c = pool.tile([128, 3, T], dt.float16, tag="c", bufs=2, name=f"c{ch}")
            nc.vector.tensor_sub(c[:], pa[:], pb[:])

            # s = |c|^2 in fp32 (squares on ACT)
            sq = pool.tile([128, 3, T], dt.float32, tag="sq", bufs=1, name=f"sq{ch}")
            nc.scalar.square(sq[:], c[:])
            s01 = pool.tile([128, T], dt.float32, tag="s01", bufs=1, name=f"s01{ch}")
            nc.vector.tensor_add(s01[:], sq[:, 0, :], sq[:, 1, :])
            s = pool.tile([128, T], dt.float32, tag="s", bufs=1, name=f"s{ch}")
            nc.vector.tensor_add(s[:], s01[:], sq[:, 2, :])

            q = pool.tile([128, T], dt.float32, tag="q", bufs=1, name=f"q{ch}")
            nc.vector.reciprocal_approx_fast(out=q[:], in_=s[:])
            # r = sqrt(q)/16 to keep r in fp16 range even for near-parallel
            # (a1, a2); the 16x/256x is folded into the Bm/Ca weights.
            r = pool.tile([128, T], dt.float16, tag="r", bufs=1, name=f"r{ch}")
            nc.scalar.activation(r[:], q[:], AF.Sqrt, scale=1.0 / 256.0)

            # cr = c * r (broadcast r over the component axis), fp16 2x
            rb = r[:].unsqueeze(1).broadcast_to([128, 3, T])
            cr = pool.tile([128, 3, T], dt.float16, tag="cr", bufs=1, name=f"cr{ch}")
            nc.vector.tensor_mul(cr[:], c[:], rb)

            # H[j] = cr_j * x  -> H[j][:, k, :] = cr_j * x_k
            H = []
            for j in range(3):
                Hj = pool.tile([128, 3, T], dt.float16, tag=f"H{j}", bufs=2,
                               name=f"H{j}_{ch}")
                crb = cr[:, j, :].unsqueeze(1).broadcast_to([128, 3, T])
                nc.vector.tensor_mul(Hj[:], crb, xin[:])
                H.append(Hj)

            # t = b3 . x = sum_k H[k][k]
            t01 = pool.tile([128, T], dt.float16, tag="t01", bufs=1, name=f"t01{ch}")
            nc.vector.tensor_add(t01[:], H[0][:, 0, :], H[1][:, 1, :])
            t = pool.tile([128, T], dt.float16, tag="t", bufs=1, name=f"t{ch}")
            nc.vector.tensor_add(t[:], t01[:], H[2][:, 2, :])

            # ct = cr * t
            tb = t[:].unsqueeze(1).broadcast_to([128, 3, T])
            ct = pool.tile([128, 3, T], dt.float16, tag="ct", bufs=2, name=f"ct{ch}")
            nc.vector.tensor_mul(ct[:], cr[:], tb)

            yout = pool.tile([128, 3, T], dt.float16, tag="yout", bufs=2,
                             name=f"yout{ch}")
            for sl in range(T // MM_FREE):
                scol = slice(sl * MM_FREE, (sl + 1) * MM_FREE)
                ps = []
                for i in range(3):
                    p_i = psum.tile([128, MM_FREE], dt.float32, tag=f"ps{i}",
                                    bufs=2, name=f"ps{i}_{ch}_{sl}")
                    ps.append(p_i)
                # weight-major issue order to minimize LDWEIGHTS
                for i in range(3):
                    nc.tensor.matmul(ps[i][:], wtile[:, 0, :], xin[:, i, scol],
                                     start=True, stop=False)
                for i in range(3):
                    nc.tensor.matmul(ps[i][:], wtile[:, 1, :], ct[:, i, scol],
                                     start=False, stop=False)
                for i in range(3):
                    a, b = AB[i]
                    nc.tensor.matmul(ps[i][:], wtile[:, 2, :], H[a][:, b, scol],
                                     start=False, stop=False)
                for i in range(3):
                    a, b = AB[i]
                    nc.tensor.matmul(ps[i][:], wtile[:, 3, :], H[b][:, a, scol],
                                     start=False, stop=True)
                for i in range(3):
                    nc.scalar.copy(yout[:, i, scol], ps[i][:])

            nc.sync.dma_start(yt.ap()[:, :, cols].rearrange("m p t -> p m t"),
                              yout[:])

    nc.compile()
    return nc


def _plane(arr2d):
    """[8192, 64] -> [128, 4096] with p = h*64+d, j = n%4096."""
    return np.ascontiguousarray(
        arr2d.reshape(2, NHALF, D).transpose(0, 2, 1).reshape(128, NHALF)
    )


def _pack_core(Jb, Xb):
    jt = np.empty((6, 128, NHALF), dtype=np.float32)
    for m, (k, cc) in enumerate(_J_PLANES):
        jt[m] = _plane(Jb[:, :, k, cc])
    xt = np.empty((3, 128, NHALF), dtype=np.float16)
    for i in range(3):
        xt[i] = _plane(Xb[:, :, i]).astype(np.float16)
    return jt, xt


def _blockdiag_T(W):
    out = np.zeros((128, 128), dtype=np.float16)
    out[:64, :64] = W.T.astype(np.float16)
    out[64:, 64:] = W.T.astype(np.float16)
    return out


def kernel(X, J, A, Bm, C):
    if "nc" not in _CACHE:
        _CACHE["nc"] = _build_nc()
    nc = _CACHE["nc"]

    X = np.asarray(X)
    J = np.asarray(J)
    # device computes cr = b3/16 (fp16-range safety), so ct carries 1/256
    # and the H cross terms carry 1/16 — compensate in the weights.
    wts = np.stack([
        _blockdiag_T(np.asarray(A)),
        _blockdiag_T(256.0 * (np.asarray(C) - np.asarray(A))),
        _blockdiag_T(16.0 * np.asarray(Bm)),
        _blockdiag_T(-16.0 * np.asarray(Bm)),
    ])

    in_maps = []
    for b in range(NCORES):
        jt, xt = _pack_core(J[b], X[b])
        in_maps.append({"jt": jt, "xt": xt, "wt": wts})

    from concourse import bass_utils
    res = bass_utils.run_bass_kernel_spmd(nc, in_maps, core_ids=list(range(NCORES)))

    Y = np.empty((B, N, F, 3), dtype=np.float32)
    for b in range(NCORES):
        yt = res.results[b]["yt"].astype(np.float32)  # [3, 128, 4096]
        Y[b] = (yt.reshape(3, 2, F, NHALF)
                  .transpose(1, 3, 2, 0)
                  .reshape(N, F, 3))
    return Y
